# revision 43
# baseline (speedup 1.0000x reference)
"""Trainium2 Bass kernel for nn_BatchProgramClassifier.

Reference computation (B=64, L=64, NPT=127, D=128, VOCAB=30000, LABELS=30):
  1. e = emb[tokens] @ Wc + bc                     per tree node
  2. h = bottom-up subtree sums of e (heap tree)   [B, L, NPT, D]
  3. enc = relu(max over nodes of h)               [B, L, D]
  4. masked single-head self-attention over L      [B, L, D]
  5. logits = (max over L) @ Wl + bl               [B, LABELS]

Sharding: data-parallel over batch, 8 batches per core across 8 cores.

Per-core device program (fp16 matmul operands, f32 PSUM accumulation):
  - dma_gather in transpose mode pulls fp16 embedding rows from HBM straight
    into D-major layout: e^T [128=D, tokens]. The software-DGE descriptor
    generation on the GpSimd Q7 cores is the kernel's critical path; each
    swdge queue runs on a dedicated core pair, so num_swdge_queues=4 with
    round-robin queue assignment keeps all 8 cores generating descriptors.
  - Column order per tree is a LEVEL-SPLIT permutation (applied host-side to
    the gather indices): [leaves(64) | lvl5(32) | ... | root | pad], with the
    children of level-l node n at positions n and n+cnt of the level-(l+1)
    region. Subtree sums then become two contiguous stride-1 half-adds per
    level and the node-max becomes pairwise halvings over aligned extents,
    which keeps every DVE op in the 2x packed perf mode (2B dtypes, unit
    stride, 4B alignment). The pad column duplicates a leaf so the max can
    run over the full 128-wide extent.
  - One Wc-stationary matmul per 512 gathered columns; the PSUM->SBUF copy on
    ACT folds the +bc bias (per-partition activation bias), writing fp16.
  - Processing is batched at 64-tree (one batch) granularity so tree sums,
    node-max, ReLU (scalar engine) and each batch's attention overlap the
    remaining gathers. The last batch processes trees per 16-tree chunk and
    its gathers are split across queues so little work trails the final
    gather.
  - Attention per batch: q/k/v matmuls into one PSUM bank with a single fused
    PSUM->SBUF copy (Wq pre-scaled by 1/sqrt(D) host-side), scores matmul,
    f16 additive mask (-30000), exp with the row-max folded in as the ACT
    bias and the row-sum accumulated in-pass (accum_out), the 1/sum scale as
    a per-partition-scaled ACT copy, attn/v transposes with one fused f16
    PSUM->SBUF copy, attn@v, Wo matmul, seq-max.
"""

import math

import numpy as np

B, L, NPT, D_TREE = 64, 64, 127, 7
VOCAB, D, LABELS = 30000, 128, 30
VPAD = 30080  # vocab padded to a multiple of 128
NRANKS = VPAD // 128
NCORES = 8
BC = B // NCORES  # batches per core
TREES = BC * L  # trees per core
CHUNK_TREES = 16  # trees per gather chunk
NCHUNKS = TREES // CHUNK_TREES
NIDX_CHUNK = CHUNK_TREES * 128
NIDX_TOTAL = TREES * 128
CHUNKS_PER_BATCH = L // CHUNK_TREES  # 4

_CACHE = {}


def _build_nc():
    import concourse.bacc as bacc
    import concourse.mybir as mybir
    import concourse.tile as tile
    from concourse.library_config import mlp

    f32 = mybir.dt.float32
    f16 = mybir.dt.float16
    nc = bacc.Bacc(
        "TRN2",
        target_bir_lowering=False,
        debug=False,
        num_devices=NCORES,
        num_swdge_queues=4,
    )

    emb_d = nc.dram_tensor("emb", [VPAD, D], f16, kind="ExternalInput")
    idx_d = nc.dram_tensor(
        "idxs", [128, NIDX_TOTAL // 16], mybir.dt.int16, kind="ExternalInput"
    )
    mask_d = nc.dram_tensor("mask", [L, BC * L], f16, kind="ExternalInput")
    wc_d = nc.dram_tensor("wc", [D, D], f32, kind="ExternalInput")
    bcv_d = nc.dram_tensor("bcv", [D, 1], f32, kind="ExternalInput")
    wq_d = nc.dram_tensor("wq", [D, D], f32, kind="ExternalInput")
    wk_d = nc.dram_tensor("wk", [D, D], f32, kind="ExternalInput")
    wv_d = nc.dram_tensor("wv", [D, D], f32, kind="ExternalInput")
    wo_d = nc.dram_tensor("wo", [D, D], f32, kind="ExternalInput")
    wl_d = nc.dram_tensor("wl", [D, LABELS], f32, kind="ExternalInput")
    blb_d = nc.dram_tensor("blb", [BC, LABELS], f32, kind="ExternalInput")
    ident_d = nc.dram_tensor("ident", [128, 128], f32, kind="ExternalInput")
    out_d = nc.dram_tensor("out", [BC, LABELS], f32, kind="ExternalOutput")

    inv_sqrt_d = 1.0 / math.sqrt(float(D))

    with tile.TileContext(nc) as tc:
        with (
            tc.tile_pool(name="const", bufs=1) as cpool,
            tc.tile_pool(name="epool", bufs=12) as epool,
            tc.tile_pool(name="eblk", bufs=2) as eblkpool,
            tc.tile_pool(name="apool", bufs=2) as apool,
            tc.tile_pool(name="tpsum", bufs=3, space="PSUM") as tpsum,
            tc.tile_pool(name="apsum", bufs=2, space="PSUM") as apsum,
            tc.tile_pool(name="spsum", bufs=2, space="PSUM") as spsum,
            tc.tile_pool(name="stsum", bufs=1, space="PSUM") as stsum,
        ):
            nc.gpsimd.load_library(mlp)

            # whole index array in one DMA on the Activation hwdge queue so
            # it runs parallel to the gpsimd library load on the sync queue
            idx_t = cpool.tile([128, NIDX_TOTAL // 16], mybir.dt.int16, tag="idxs")
            nc.scalar.dma_start(out=idx_t[:], in_=idx_d[:])

            # embedding table resident in SBUF: token t at partition t%128,
            # rank t//128 (256B stripe per rank). SBUF-source gathers drain
            # their descriptor rings at xbar speed instead of random-read
            # HBM latency.
            emb_sb = cpool.tile([128, NRANKS, D], f16, tag="embsb")
            nc.sync.dma_start(
                out=emb_sb[:],
                in_=emb_d[:].rearrange("(r p) d -> p r d", p=128),
            )

            def load_const(dram, shape, dtype):
                t = cpool.tile(shape, dtype, tag=dram.name)
                if dtype == dram.dtype:
                    nc.scalar.dma_start(out=t[:], in_=dram[:])
                else:
                    raw = cpool.tile(shape, dram.dtype, tag=dram.name + "_raw")
                    nc.scalar.dma_start(out=raw[:], in_=dram[:])
                    nc.scalar.copy(out=t[:], in_=raw[:])
                return t

            wc_t = load_const(wc_d, [D, D], f16)
            bcv_t = load_const(bcv_d, [D, 1], f32)
            wq_t = load_const(wq_d, [D, D], f16)
            wk_t = load_const(wk_d, [D, D], f16)
            wv_t = load_const(wv_d, [D, D], f16)
            wo_t = load_const(wo_d, [D, D], f16)
            wl_t = load_const(wl_d, [D, LABELS], f16)
            blb_t = load_const(blb_d, [BC, LABELS], f32)
            ident_t = load_const(ident_d, [128, 128], f16)
            # additive mask (0 / -30000, f16-safe) computed host-side
            nmask = load_const(mask_d, [L, BC * L], f16)

            enc = cpool.tile([D, TREES], f16, tag="enc")  # enc^T, col = b*64+l
            pooled = cpool.tile([D, BC], f16, tag="pooled")

            idx_cols = NIDX_CHUNK // 16

            def tree_block(ebv, encb, nt):
                # Bottom-up subtree sums in the level-split layout: each
                # tree's 128 cols are [leaves(64) | lvl5(32) | ... | root | pad],
                # children of lvl-l node n sit at positions n and n+cnt of the
                # lvl-(l+1) region, so each level is two contiguous stride-1
                # half-adds (DVE 2x packed mode).
                for lvl in range(D_TREE - 2, -1, -1):
                    cnt = 2**lvl
                    sl = 128 - 2 ** (lvl + 1)
                    sc = 128 - 2 ** (lvl + 2)
                    nc.vector.tensor_add(
                        out=ebv[:, :, sl : sl + cnt],
                        in0=ebv[:, :, sl : sl + cnt],
                        in1=ebv[:, :, sc : sc + cnt],
                    )
                    nc.vector.tensor_add(
                        out=ebv[:, :, sl : sl + cnt],
                        in0=ebv[:, :, sl : sl + cnt],
                        in1=ebv[:, :, sc + cnt : sc + 2 * cnt],
                    )
                # node-max as pairwise halvings (2x mode; pad col duplicates
                # a leaf so the full 128-wide extent is safe to max over)
                mx = apool.tile([128, nt, 64], f16, tag=f"mx{nt}")
                nc.vector.tensor_max(mx[:], ebv[:, :, 0:64], ebv[:, :, 64:128])
                w = 32
                while w >= 2:
                    nc.vector.tensor_max(
                        mx[:, :, 0:w], mx[:, :, 0:w], mx[:, :, w : 2 * w]
                    )
                    w //= 2
                nc.vector.tensor_max(encb, mx[:, :, 0], mx[:, :, 1])
                # ReLU (max with the zero background) on the scalar engine
                nc.scalar.activation(encb, encb, mybir.ActivationFunctionType.Relu)

            for b in range(BC):
                eb = eblkpool.tile([128, L * 128], f16, tag="eb")
                ebv = eb.rearrange("p (t n) -> p t n", n=128)
                # last batch runs tree+max per 16-tree chunk so less of the
                # chain trails the final gathers
                fine = b >= BC - 1
                for k in range(CHUNKS_PER_BATCH):
                    c = b * CHUNKS_PER_BATCH + k
                    et = epool.tile([128, 1, NIDX_CHUNK], f16, tag="et")
                    last = c >= NCHUNKS - 4
                    nsub = 2 if last else 1
                    si = NIDX_CHUNK // nsub
                    sc_ = idx_cols // nsub
                    for sj in range(nsub):
                        nc.gpsimd.dma_gather(
                            et[:, :, sj * si : (sj + 1) * si],
                            emb_sb[:],
                            idx_t[
                                :,
                                c * idx_cols + sj * sc_ : c * idx_cols + (sj + 1) * sc_,
                            ],
                            si,
                            si,
                            D,
                            transpose=True,
                            single_packet=False,
                            queue_num=(2 * (c % 2) + sj) if last else c % 4,
                            sbuf_tokens_per_rank=128,
                            sbuf_free_dim_per_rank=D * 2,
                        )
                    for j in range(NIDX_CHUNK // 512):
                        pp = tpsum.tile([128, 512], f32, tag="pp")
                        nc.tensor.matmul(
                            pp[:],
                            lhsT=wc_t[:],
                            rhs=et[:, 0, j * 512 : (j + 1) * 512],
                            start=True,
                            stop=True,
                        )
                        off = k * NIDX_CHUNK + j * 512
                        # PSUM->SBUF copy with the +bc bias folded in
                        nc.scalar.activation(
                            eb[:, off : off + 512],
                            pp[:],
                            mybir.ActivationFunctionType.Identity,
                            bias=bcv_t[:],
                            scale=1.0,
                        )
                    if fine:
                        t0 = k * CHUNK_TREES
                        tree_block(
                            ebv[:, t0 : t0 + CHUNK_TREES, :],
                            enc[:, b * L + t0 : b * L + t0 + CHUNK_TREES],
                            CHUNK_TREES,
                        )
                if not fine:
                    tree_block(ebv, enc[:, b * L : (b + 1) * L], L)
                encb = enc[:, b * L : (b + 1) * L]

                # ---- attention for this batch (Wq pre-scaled by 1/sqrt(D)) ----
                ab = apsum.tile([D, 512], f32, tag="ab")
                nc.tensor.matmul(ab[:, 0:L], lhsT=wq_t[:], rhs=encb, start=True, stop=True)
                nc.tensor.matmul(
                    ab[:, L : 2 * L], lhsT=wk_t[:], rhs=encb, start=True, stop=True
                )
                nc.tensor.matmul(
                    ab[:, 2 * L : 3 * L], lhsT=wv_t[:], rhs=encb, start=True, stop=True
                )
                qkvs = apool.tile([D, 3 * L], f16, tag="qkvs")
                nc.scalar.copy(out=qkvs[:], in_=ab[:, 0 : 3 * L])
                qs = qkvs[:, 0:L]
                ks = qkvs[:, L : 2 * L]
                vs = qkvs[:, 2 * L : 3 * L]

                sp = spsum.tile([L, 512], f32, tag="sp")
                nc.tensor.matmul(sp[:, 0:L], lhsT=qs, rhs=ks, start=True, stop=True)
                sm = apool.tile([L, L], f16, tag="sm")
                nc.vector.tensor_add(
                    out=sm[:], in0=sp[:, 0:L], in1=nmask[:, b * L : (b + 1) * L]
                )
                smv = sm.rearrange("q (o k) -> q o k", k=L)
                nrmax = apool.tile([L, 1], f32, tag="nrmax")
                nc.vector.reduce_max(
                    out=nrmax[:], in_=smv, axis=mybir.AxisListType.X, negate=True
                )
                # ex = exp(sm - rowmax) with the row sum accumulated in-pass
                ex = apool.tile([L, L], f32, tag="ex")
                rsum = apool.tile([L, 1], f32, tag="rsum")
                nc.scalar.activation(
                    ex[:],
                    sm[:],
                    mybir.ActivationFunctionType.Exp,
                    bias=nrmax[:],
                    scale=1.0,
                    accum_out=rsum[:],
                )
                rinv = apool.tile([L, 1], f32, tag="rinv")
                nc.vector.reciprocal(rinv[:], rsum[:])
                # attn = ex * rinv as a per-partition-scaled scalar-engine copy
                attn = apool.tile([L, L], f16, tag="attn")
                nc.scalar.activation(
                    attn[:],
                    ex[:],
                    mybir.ActivationFunctionType.Identity,
                    scale=rinv[:],
                )

                # transposes and attn@v (one fused PSUM->SBUF copy for both)
                spt = stsum.tile([L, 256], f16, tag="spt")
                nc.tensor.transpose(spt[:, 0:L], attn[:], ident_t[:L, :L])
                nc.tensor.transpose(spt[:, L : L + D], vs, ident_t[:])
                avs = apool.tile([L, 3 * L], f16, tag="avs")
                nc.scalar.copy(out=avs[:], in_=spt[:, 0 : 3 * L])
                ats = avs[:, 0:L]
                vts = avs[:, L : 3 * L]
                nc.tensor.matmul(
                    ab[:, 3 * L : 4 * L], lhsT=vts, rhs=ats, start=True, stop=True
                )
                os_t = apool.tile([D, L], f16, tag="os")
                nc.scalar.copy(out=os_t[:], in_=ab[:, 3 * L : 4 * L])
                nc.tensor.matmul(
                    ab[:, 4 * L : 5 * L], lhsT=wo_t[:], rhs=os_t[:], start=True, stop=True
                )
                nc.vector.reduce_max(
                    out=pooled[:, b : b + 1],
                    in_=ab[:, 4 * L : 5 * L].rearrange("d (o l) -> d o l", l=L),
                    axis=mybir.AxisListType.X,
                )

            # ---- logits ----
            lgp = spsum.tile([L, 512], f32, tag="sp")
            nc.tensor.matmul(
                lgp[0:BC, 0:LABELS], lhsT=pooled[:], rhs=wl_t[:], start=True, stop=True
            )
            outs = apool.tile([BC, LABELS], f32, tag="outs")
            nc.vector.tensor_add(out=outs[:], in0=lgp[0:BC, 0:LABELS], in1=blb_t[:])
            nc.sync.dma_start(out=out_d[:], in_=outs[:])

    nc.compile()
    return nc


def _get_nc():
    if "nc" not in _CACHE:
        _CACHE["nc"] = _build_nc()
    return _CACHE["nc"]


def kernel(tokens, mask, emb, Wc, bc, Wq, Wk, Wv, Wo, Wl, bl, _trace=False):
    from concourse.bass_utils import run_bass_kernel_spmd

    tokens = np.asarray(tokens)
    mask = np.asarray(mask)
    emb16 = np.asarray(emb, dtype=np.float32).astype(np.float16)
    emb16 = np.concatenate(
        [emb16, np.zeros((VPAD - VOCAB, D), np.float16)], axis=0
    )

    blb = np.tile(np.asarray(bl, np.float32)[None, :], (BC, 1))

    common = {
        "emb": emb16,
        "wc": np.asarray(Wc, np.float32),
        "bcv": np.asarray(bc, np.float32).reshape(D, 1),
        "wq": np.asarray(Wq, np.float32) / math.sqrt(float(D)),
        "wk": np.asarray(Wk, np.float32),
        "wv": np.asarray(Wv, np.float32),
        "wo": np.asarray(Wo, np.float32),
        "wl": np.asarray(Wl, np.float32),
        "blb": blb,
        "ident": np.eye(128, dtype=np.float32),
    }

    # level-split column order: [leaves(64) | lvl5(32) | ... | root], with
    # children of position n in level l at positions n and n+2**l of level l+1
    lvl_nodes = [[0]]
    for _ in range(D_TREE - 1):
        prev = lvl_nodes[-1]
        lvl_nodes.append([2 * k + 1 for k in prev] + [2 * k + 2 for k in prev])
    order = np.array([k for lvl in reversed(lvl_nodes) for k in lvl], np.int64)

    in_maps = []
    for c in range(NCORES):
        tok_c = np.asarray(tokens[c * BC : (c + 1) * BC]).reshape(TREES, NPT)
        tok_c = tok_c[:, order]
        # pad node duplicates a leaf: the max over the padded 128-wide
        # extent equals the max over the 127 real subtree-root snapshots
        idx_lin = np.concatenate([tok_c, tok_c[:, 0:1]], axis=1).reshape(-1)
        idx_arr = np.tile(
            idx_lin.astype(np.int16).reshape(-1, 16).T, (8, 1)
        )  # [128, NIDX_TOTAL/16]
        mask_c = (
            np.asarray(mask[c * BC : (c + 1) * BC], np.int32)
            .transpose(1, 0, 2)
            .reshape(L, BC * L)
        )
        mask_c = np.where(mask_c > 0, np.float16(0), np.float16(-30000))
        in_maps.append({**common, "idxs": idx_arr, "mask": mask_c})

    nc = _get_nc()
    res = run_bass_kernel_spmd(
        nc, in_maps, core_ids=list(range(NCORES)), trace=_trace
    )
    out = np.concatenate([r["out"] for r in res.results], axis=0)  # [B, LABELS]
    if _trace:
        return out, res
    return out


# revision 44
# speedup vs baseline: 1.6117x; 1.6117x over previous
"""Trainium2 Bass kernel for nn_BatchProgramClassifier.

Reference computation (B=64, L=64, NPT=127, D=128, VOCAB=30000, LABELS=30):
  1. e = emb[tokens] @ Wc + bc                     per tree node
  2. h = bottom-up subtree sums of e (heap tree)   [B, L, NPT, D]
  3. enc = relu(max over nodes of h)               [B, L, D]
  4. masked single-head self-attention over L      [B, L, D]
  5. logits = (max over L) @ Wl + bl               [B, LABELS]

Sharding: data-parallel over batch, 8 batches per core across 8 cores.

Per-core device program (fp16 matmul operands, f32 PSUM accumulation):
  - dma_gather in transpose mode pulls fp16 embedding rows from HBM straight
    into D-major layout: e^T [128=D, tokens]. The software-DGE descriptor
    generation on the GpSimd Q7 cores is the kernel's critical path; each
    swdge queue runs on a dedicated core pair, so num_swdge_queues=4 with
    round-robin queue assignment keeps all 8 cores generating descriptors.
  - Column order per tree is a LEVEL-SPLIT permutation (applied host-side to
    the gather indices): [leaves(64) | lvl5(32) | ... | root | pad], with the
    children of level-l node n at positions n and n+cnt of the level-(l+1)
    region. Subtree sums then become two contiguous stride-1 half-adds per
    level and the node-max becomes pairwise halvings over aligned extents,
    which keeps every DVE op in the 2x packed perf mode (2B dtypes, unit
    stride, 4B alignment). The pad column duplicates a leaf so the max can
    run over the full 128-wide extent.
  - One Wc-stationary matmul per 512 gathered columns; the PSUM->SBUF copy on
    ACT folds the +bc bias (per-partition activation bias), writing fp16.
  - Processing is batched at 64-tree (one batch) granularity so tree sums,
    node-max, ReLU (scalar engine) and each batch's attention overlap the
    remaining gathers. The last batch processes trees per 16-tree chunk and
    its gathers are split across queues so little work trails the final
    gather.
  - Attention per batch: q/k/v matmuls into one PSUM bank with a single fused
    PSUM->SBUF copy (Wq pre-scaled by 1/sqrt(D) host-side), scores matmul,
    f16 additive mask (-30000), exp with the row-max folded in as the ACT
    bias and the row-sum accumulated in-pass (accum_out), the 1/sum scale as
    a per-partition-scaled ACT copy, attn/v transposes with one fused f16
    PSUM->SBUF copy, attn@v, Wo matmul, seq-max.
"""

import math

import numpy as np

B, L, NPT, D_TREE = 64, 64, 127, 7
VOCAB, D, LABELS = 30000, 128, 30
VPAD = 30080  # vocab padded to a multiple of 128
NRANKS = VPAD // 128
NCORES = 8
BC = B // NCORES  # batches per core
TREES = BC * L  # trees per core
CHUNK_TREES = 16  # trees per gather chunk
NCHUNKS = TREES // CHUNK_TREES
NIDX_CHUNK = CHUNK_TREES * 128
NIDX_TOTAL = TREES * 128
CHUNKS_PER_BATCH = L // CHUNK_TREES  # 4

_CACHE = {}


def _build_nc():
    import concourse.bacc as bacc
    import concourse.mybir as mybir
    import concourse.tile as tile
    from concourse.library_config import mlp

    f32 = mybir.dt.float32
    f16 = mybir.dt.float16
    nc = bacc.Bacc(
        "TRN2",
        target_bir_lowering=False,
        debug=False,
        num_devices=NCORES,
        num_swdge_queues=4,
    )

    emb_d = nc.dram_tensor("emb", [VPAD, D], f16, kind="ExternalInput")
    idx_d = nc.dram_tensor(
        "idxs", [128, NIDX_TOTAL // 16], mybir.dt.int16, kind="ExternalInput"
    )
    mask_d = nc.dram_tensor("mask", [L, BC * L], f16, kind="ExternalInput")
    wc_d = nc.dram_tensor("wc", [D, D], f32, kind="ExternalInput")
    bcv_d = nc.dram_tensor("bcv", [D, 1], f32, kind="ExternalInput")
    wq_d = nc.dram_tensor("wq", [D, D], f32, kind="ExternalInput")
    wk_d = nc.dram_tensor("wk", [D, D], f32, kind="ExternalInput")
    wv_d = nc.dram_tensor("wv", [D, D], f32, kind="ExternalInput")
    wo_d = nc.dram_tensor("wo", [D, D], f32, kind="ExternalInput")
    wl_d = nc.dram_tensor("wl", [D, LABELS], f32, kind="ExternalInput")
    blb_d = nc.dram_tensor("blb", [BC, LABELS], f32, kind="ExternalInput")
    ident_d = nc.dram_tensor("ident", [128, 128], f32, kind="ExternalInput")
    out_d = nc.dram_tensor("out", [BC, LABELS], f32, kind="ExternalOutput")

    inv_sqrt_d = 1.0 / math.sqrt(float(D))

    with tile.TileContext(nc) as tc:
        with (
            tc.tile_pool(name="const", bufs=1) as cpool,
            tc.tile_pool(name="epool", bufs=12) as epool,
            tc.tile_pool(name="eblk", bufs=2) as eblkpool,
            tc.tile_pool(name="apool", bufs=2) as apool,
            tc.tile_pool(name="tpsum", bufs=3, space="PSUM") as tpsum,
            tc.tile_pool(name="apsum", bufs=2, space="PSUM") as apsum,
            tc.tile_pool(name="spsum", bufs=2, space="PSUM") as spsum,
            tc.tile_pool(name="stsum", bufs=1, space="PSUM") as stsum,
        ):
            nc.gpsimd.load_library(mlp)

            # whole index array in one DMA on the Activation hwdge queue so
            # it runs parallel to the gpsimd library load on the sync queue
            idx_t = cpool.tile([128, NIDX_TOTAL // 16], mybir.dt.int16, tag="idxs")
            nc.scalar.dma_start(out=idx_t[:], in_=idx_d[:])

            def load_const(dram, shape, dtype):
                t = cpool.tile(shape, dtype, tag=dram.name)
                if dtype == dram.dtype:
                    nc.scalar.dma_start(out=t[:], in_=dram[:])
                else:
                    raw = cpool.tile(shape, dram.dtype, tag=dram.name + "_raw")
                    nc.scalar.dma_start(out=raw[:], in_=dram[:])
                    nc.scalar.copy(out=t[:], in_=raw[:])
                return t

            wc_t = load_const(wc_d, [D, D], f16)
            bcv_t = load_const(bcv_d, [D, 1], f32)
            wq_t = load_const(wq_d, [D, D], f16)
            wk_t = load_const(wk_d, [D, D], f16)
            wv_t = load_const(wv_d, [D, D], f16)
            wo_t = load_const(wo_d, [D, D], f16)
            wl_t = load_const(wl_d, [D, LABELS], f16)
            blb_t = load_const(blb_d, [BC, LABELS], f32)
            ident_t = load_const(ident_d, [128, 128], f16)
            # additive mask (0 / -30000, f16-safe) computed host-side
            nmask = load_const(mask_d, [L, BC * L], f16)

            enc = cpool.tile([D, TREES], f16, tag="enc")  # enc^T, col = b*64+l
            pooled = cpool.tile([D, BC], f16, tag="pooled")

            idx_cols = NIDX_CHUNK // 16

            def tree_block(ebv, encb, nt):
                # Bottom-up subtree sums in the level-split layout: each
                # tree's 128 cols are [leaves(64) | lvl5(32) | ... | root | pad],
                # children of lvl-l node n sit at positions n and n+cnt of the
                # lvl-(l+1) region, so each level is two contiguous stride-1
                # half-adds (DVE 2x packed mode).
                for lvl in range(D_TREE - 2, -1, -1):
                    cnt = 2**lvl
                    sl = 128 - 2 ** (lvl + 1)
                    sc = 128 - 2 ** (lvl + 2)
                    nc.vector.tensor_add(
                        out=ebv[:, :, sl : sl + cnt],
                        in0=ebv[:, :, sl : sl + cnt],
                        in1=ebv[:, :, sc : sc + cnt],
                    )
                    nc.vector.tensor_add(
                        out=ebv[:, :, sl : sl + cnt],
                        in0=ebv[:, :, sl : sl + cnt],
                        in1=ebv[:, :, sc + cnt : sc + 2 * cnt],
                    )
                # node-max as pairwise halvings (2x mode; pad col duplicates
                # a leaf so the full 128-wide extent is safe to max over)
                mx = apool.tile([128, nt, 64], f16, tag=f"mx{nt}")
                nc.vector.tensor_max(mx[:], ebv[:, :, 0:64], ebv[:, :, 64:128])
                w = 32
                while w >= 2:
                    nc.vector.tensor_max(
                        mx[:, :, 0:w], mx[:, :, 0:w], mx[:, :, w : 2 * w]
                    )
                    w //= 2
                nc.vector.tensor_max(encb, mx[:, :, 0], mx[:, :, 1])
                # ReLU (max with the zero background) on the scalar engine
                nc.scalar.activation(encb, encb, mybir.ActivationFunctionType.Relu)

            for b in range(BC):
                eb = eblkpool.tile([128, L * 128], f16, tag="eb")
                ebv = eb.rearrange("p (t n) -> p t n", n=128)
                # last batch runs tree+max per 16-tree chunk so less of the
                # chain trails the final gathers
                fine = b >= BC - 1
                for k in range(CHUNKS_PER_BATCH):
                    c = b * CHUNKS_PER_BATCH + k
                    et = epool.tile([128, 1, NIDX_CHUNK], f16, tag="et")
                    last = c >= NCHUNKS - 4
                    nsub = 2 if last else 1
                    si = NIDX_CHUNK // nsub
                    sc_ = idx_cols // nsub
                    for sj in range(nsub):
                        nc.gpsimd.dma_gather(
                            et[:, :, sj * si : (sj + 1) * si],
                            emb_d[:],
                            idx_t[
                                :,
                                c * idx_cols + sj * sc_ : c * idx_cols + (sj + 1) * sc_,
                            ],
                            si,
                            si,
                            D,
                            transpose=True,
                            single_packet=False,
                            queue_num=(2 * (c % 2) + sj) if last else c % 4,
                        )
                    for j in range(NIDX_CHUNK // 512):
                        pp = tpsum.tile([128, 512], f32, tag="pp")
                        nc.tensor.matmul(
                            pp[:],
                            lhsT=wc_t[:],
                            rhs=et[:, 0, j * 512 : (j + 1) * 512],
                            start=True,
                            stop=True,
                        )
                        off = k * NIDX_CHUNK + j * 512
                        # PSUM->SBUF copy with the +bc bias folded in
                        nc.scalar.activation(
                            eb[:, off : off + 512],
                            pp[:],
                            mybir.ActivationFunctionType.Identity,
                            bias=bcv_t[:],
                            scale=1.0,
                        )
                    if fine:
                        t0 = k * CHUNK_TREES
                        tree_block(
                            ebv[:, t0 : t0 + CHUNK_TREES, :],
                            enc[:, b * L + t0 : b * L + t0 + CHUNK_TREES],
                            CHUNK_TREES,
                        )
                if not fine:
                    tree_block(ebv, enc[:, b * L : (b + 1) * L], L)
                encb = enc[:, b * L : (b + 1) * L]

                # ---- attention for this batch (Wq pre-scaled by 1/sqrt(D)) ----
                ab = apsum.tile([D, 512], f32, tag="ab")
                nc.tensor.matmul(ab[:, 0:L], lhsT=wq_t[:], rhs=encb, start=True, stop=True)
                nc.tensor.matmul(
                    ab[:, L : 2 * L], lhsT=wk_t[:], rhs=encb, start=True, stop=True
                )
                nc.tensor.matmul(
                    ab[:, 2 * L : 3 * L], lhsT=wv_t[:], rhs=encb, start=True, stop=True
                )
                qkvs = apool.tile([D, 3 * L], f16, tag="qkvs")
                nc.scalar.copy(out=qkvs[:], in_=ab[:, 0 : 3 * L])
                qs = qkvs[:, 0:L]
                ks = qkvs[:, L : 2 * L]
                vs = qkvs[:, 2 * L : 3 * L]

                sp = spsum.tile([L, 512], f32, tag="sp")
                nc.tensor.matmul(sp[:, 0:L], lhsT=qs, rhs=ks, start=True, stop=True)
                sm = apool.tile([L, L], f16, tag="sm")
                nc.vector.tensor_add(
                    out=sm[:], in0=sp[:, 0:L], in1=nmask[:, b * L : (b + 1) * L]
                )
                smv = sm.rearrange("q (o k) -> q o k", k=L)
                nrmax = apool.tile([L, 1], f32, tag="nrmax")
                nc.vector.reduce_max(
                    out=nrmax[:], in_=smv, axis=mybir.AxisListType.X, negate=True
                )
                # ex = exp(sm - rowmax) with the row sum accumulated in-pass
                ex = apool.tile([L, L], f32, tag="ex")
                rsum = apool.tile([L, 1], f32, tag="rsum")
                nc.scalar.activation(
                    ex[:],
                    sm[:],
                    mybir.ActivationFunctionType.Exp,
                    bias=nrmax[:],
                    scale=1.0,
                    accum_out=rsum[:],
                )
                rinv = apool.tile([L, 1], f32, tag="rinv")
                nc.vector.reciprocal(rinv[:], rsum[:])
                # attn = ex * rinv as a per-partition-scaled scalar-engine copy
                attn = apool.tile([L, L], f16, tag="attn")
                nc.scalar.activation(
                    attn[:],
                    ex[:],
                    mybir.ActivationFunctionType.Identity,
                    scale=rinv[:],
                )

                # transposes and attn@v (one fused PSUM->SBUF copy for both)
                spt = stsum.tile([L, 256], f16, tag="spt")
                nc.tensor.transpose(spt[:, 0:L], attn[:], ident_t[:L, :L])
                nc.tensor.transpose(spt[:, L : L + D], vs, ident_t[:])
                avs = apool.tile([L, 3 * L], f16, tag="avs")
                nc.scalar.copy(out=avs[:], in_=spt[:, 0 : 3 * L])
                ats = avs[:, 0:L]
                vts = avs[:, L : 3 * L]
                nc.tensor.matmul(
                    ab[:, 3 * L : 4 * L], lhsT=vts, rhs=ats, start=True, stop=True
                )
                os_t = apool.tile([D, L], f16, tag="os")
                nc.scalar.copy(out=os_t[:], in_=ab[:, 3 * L : 4 * L])
                nc.tensor.matmul(
                    ab[:, 4 * L : 5 * L], lhsT=wo_t[:], rhs=os_t[:], start=True, stop=True
                )
                nc.vector.reduce_max(
                    out=pooled[:, b : b + 1],
                    in_=ab[:, 4 * L : 5 * L].rearrange("d (o l) -> d o l", l=L),
                    axis=mybir.AxisListType.X,
                )

            # ---- logits ----
            lgp = spsum.tile([L, 512], f32, tag="sp")
            nc.tensor.matmul(
                lgp[0:BC, 0:LABELS], lhsT=pooled[:], rhs=wl_t[:], start=True, stop=True
            )
            outs = apool.tile([BC, LABELS], f32, tag="outs")
            nc.vector.tensor_add(out=outs[:], in0=lgp[0:BC, 0:LABELS], in1=blb_t[:])
            nc.sync.dma_start(out=out_d[:], in_=outs[:])

    nc.compile()
    return nc


def _get_nc():
    if "nc" not in _CACHE:
        _CACHE["nc"] = _build_nc()
    return _CACHE["nc"]


def kernel(tokens, mask, emb, Wc, bc, Wq, Wk, Wv, Wo, Wl, bl, _trace=False):
    from concourse.bass_utils import run_bass_kernel_spmd

    tokens = np.asarray(tokens)
    mask = np.asarray(mask)
    emb16 = np.asarray(emb, dtype=np.float32).astype(np.float16)
    emb16 = np.concatenate(
        [emb16, np.zeros((VPAD - VOCAB, D), np.float16)], axis=0
    )

    blb = np.tile(np.asarray(bl, np.float32)[None, :], (BC, 1))

    common = {
        "emb": emb16,
        "wc": np.asarray(Wc, np.float32),
        "bcv": np.asarray(bc, np.float32).reshape(D, 1),
        "wq": np.asarray(Wq, np.float32) / math.sqrt(float(D)),
        "wk": np.asarray(Wk, np.float32),
        "wv": np.asarray(Wv, np.float32),
        "wo": np.asarray(Wo, np.float32),
        "wl": np.asarray(Wl, np.float32),
        "blb": blb,
        "ident": np.eye(128, dtype=np.float32),
    }

    # level-split column order: [leaves(64) | lvl5(32) | ... | root], with
    # children of position n in level l at positions n and n+2**l of level l+1
    lvl_nodes = [[0]]
    for _ in range(D_TREE - 1):
        prev = lvl_nodes[-1]
        lvl_nodes.append([2 * k + 1 for k in prev] + [2 * k + 2 for k in prev])
    order = np.array([k for lvl in reversed(lvl_nodes) for k in lvl], np.int64)

    in_maps = []
    for c in range(NCORES):
        tok_c = np.asarray(tokens[c * BC : (c + 1) * BC]).reshape(TREES, NPT)
        tok_c = tok_c[:, order]
        # pad node duplicates a leaf: the max over the padded 128-wide
        # extent equals the max over the 127 real subtree-root snapshots
        idx_lin = np.concatenate([tok_c, tok_c[:, 0:1]], axis=1).reshape(-1)
        idx_arr = np.tile(
            idx_lin.astype(np.int16).reshape(-1, 16).T, (8, 1)
        )  # [128, NIDX_TOTAL/16]
        mask_c = (
            np.asarray(mask[c * BC : (c + 1) * BC], np.int32)
            .transpose(1, 0, 2)
            .reshape(L, BC * L)
        )
        mask_c = np.where(mask_c > 0, np.float16(0), np.float16(-30000))
        in_maps.append({**common, "idxs": idx_arr, "mask": mask_c})

    nc = _get_nc()
    res = run_bass_kernel_spmd(
        nc, in_maps, core_ids=list(range(NCORES)), trace=_trace
    )
    out = np.concatenate([r["out"] for r in res.results], axis=0)  # [B, LABELS]
    if _trace:
        return out, res
    return out


# revision 45
# speedup vs baseline: 1.6122x; 1.0003x over previous
"""Trainium2 Bass kernel for nn_BatchProgramClassifier.

Reference computation (B=64, L=64, NPT=127, D=128, VOCAB=30000, LABELS=30):
  1. e = emb[tokens] @ Wc + bc                     per tree node
  2. h = bottom-up subtree sums of e (heap tree)   [B, L, NPT, D]
  3. enc = relu(max over nodes of h)               [B, L, D]
  4. masked single-head self-attention over L      [B, L, D]
  5. logits = (max over L) @ Wl + bl               [B, LABELS]

Sharding: data-parallel over batch, 8 batches per core across 8 cores.

Per-core device program (fp16 matmul operands, f32 PSUM accumulation):
  - dma_gather in transpose mode pulls fp16 embedding rows from HBM straight
    into D-major layout: e^T [128=D, tokens]. The software-DGE descriptor
    generation on the GpSimd Q7 cores is the kernel's critical path; each
    swdge queue runs on a dedicated core pair, so num_swdge_queues=4 with
    round-robin queue assignment keeps all 8 cores generating descriptors.
  - Column order per tree is a LEVEL-SPLIT permutation (applied host-side to
    the gather indices): [leaves(64) | lvl5(32) | ... | root | pad], with the
    children of level-l node n at positions n and n+cnt of the level-(l+1)
    region. Subtree sums then become two contiguous stride-1 half-adds per
    level and the node-max becomes pairwise halvings over aligned extents,
    which keeps every DVE op in the 2x packed perf mode (2B dtypes, unit
    stride, 4B alignment). The pad column duplicates a leaf so the max can
    run over the full 128-wide extent.
  - One Wc-stationary matmul per 512 gathered columns; the PSUM->SBUF copy on
    ACT folds the +bc bias (per-partition activation bias), writing fp16.
  - Processing is batched at 64-tree (one batch) granularity so tree sums,
    node-max, ReLU (scalar engine) and each batch's attention overlap the
    remaining gathers. The last batch processes trees per 16-tree chunk and
    its gathers are split across queues so little work trails the final
    gather.
  - Attention per batch: q/k/v matmuls into one PSUM bank with a single fused
    PSUM->SBUF copy (Wq pre-scaled by 1/sqrt(D) host-side), scores matmul,
    f16 additive mask (-30000), exp with the row-max folded in as the ACT
    bias and the row-sum accumulated in-pass (accum_out), the 1/sum scale as
    a per-partition-scaled ACT copy, attn/v transposes with one fused f16
    PSUM->SBUF copy, attn@v, Wo matmul, seq-max.
"""

import math

import numpy as np

B, L, NPT, D_TREE = 64, 64, 127, 7
VOCAB, D, LABELS = 30000, 128, 30
VPAD = 30080  # vocab padded to a multiple of 128
NRANKS = VPAD // 128
NCORES = 8
BC = B // NCORES  # batches per core
TREES = BC * L  # trees per core
CHUNK_TREES = 16  # trees per gather chunk
NCHUNKS = TREES // CHUNK_TREES
NIDX_CHUNK = CHUNK_TREES * 128
NIDX_TOTAL = TREES * 128
CHUNKS_PER_BATCH = L // CHUNK_TREES  # 4

_CACHE = {}


def _build_nc():
    import concourse.bacc as bacc
    import concourse.mybir as mybir
    import concourse.tile as tile
    from concourse.library_config import mlp

    f32 = mybir.dt.float32
    f16 = mybir.dt.float16
    nc = bacc.Bacc(
        "TRN2",
        target_bir_lowering=False,
        debug=False,
        num_devices=NCORES,
        num_swdge_queues=4,
    )

    emb_d = nc.dram_tensor("emb", [VPAD, D], f16, kind="ExternalInput")
    idx_d = nc.dram_tensor(
        "idxs", [128, NIDX_TOTAL // 16], mybir.dt.int16, kind="ExternalInput"
    )
    mask_d = nc.dram_tensor("mask", [L, BC * L], f16, kind="ExternalInput")
    wc_d = nc.dram_tensor("wc", [D, D], f32, kind="ExternalInput")
    bcv_d = nc.dram_tensor("bcv", [D, 1], f32, kind="ExternalInput")
    wq_d = nc.dram_tensor("wq", [D, D], f32, kind="ExternalInput")
    wk_d = nc.dram_tensor("wk", [D, D], f32, kind="ExternalInput")
    wv_d = nc.dram_tensor("wv", [D, D], f32, kind="ExternalInput")
    wo_d = nc.dram_tensor("wo", [D, D], f32, kind="ExternalInput")
    wl_d = nc.dram_tensor("wl", [D, LABELS], f32, kind="ExternalInput")
    blb_d = nc.dram_tensor("blb", [BC, LABELS], f32, kind="ExternalInput")
    ident_d = nc.dram_tensor("ident", [128, 128], f32, kind="ExternalInput")
    out_d = nc.dram_tensor("out", [BC, LABELS], f32, kind="ExternalOutput")

    inv_sqrt_d = 1.0 / math.sqrt(float(D))

    with tile.TileContext(nc) as tc:
        with (
            tc.tile_pool(name="const", bufs=1) as cpool,
            tc.tile_pool(name="epool", bufs=12) as epool,
            tc.tile_pool(name="eblk", bufs=3) as eblkpool,
            tc.tile_pool(name="apool", bufs=2) as apool,
            tc.tile_pool(name="tpsum", bufs=3, space="PSUM") as tpsum,
            tc.tile_pool(name="apsum", bufs=2, space="PSUM") as apsum,
            tc.tile_pool(name="spsum", bufs=2, space="PSUM") as spsum,
            tc.tile_pool(name="stsum", bufs=1, space="PSUM") as stsum,
        ):
            nc.gpsimd.load_library(mlp)

            # whole index array in one DMA on the Activation hwdge queue so
            # it runs parallel to the gpsimd library load on the sync queue
            idx_t = cpool.tile([128, NIDX_TOTAL // 16], mybir.dt.int16, tag="idxs")
            nc.scalar.dma_start(out=idx_t[:], in_=idx_d[:])

            def load_const(dram, shape, dtype):
                t = cpool.tile(shape, dtype, tag=dram.name)
                if dtype == dram.dtype:
                    nc.scalar.dma_start(out=t[:], in_=dram[:])
                else:
                    raw = cpool.tile(shape, dram.dtype, tag=dram.name + "_raw")
                    nc.scalar.dma_start(out=raw[:], in_=dram[:])
                    nc.scalar.copy(out=t[:], in_=raw[:])
                return t

            wc_t = load_const(wc_d, [D, D], f16)
            bcv_t = load_const(bcv_d, [D, 1], f32)
            wq_t = load_const(wq_d, [D, D], f16)
            wk_t = load_const(wk_d, [D, D], f16)
            wv_t = load_const(wv_d, [D, D], f16)
            wo_t = load_const(wo_d, [D, D], f16)
            wl_t = load_const(wl_d, [D, LABELS], f16)
            blb_t = load_const(blb_d, [BC, LABELS], f32)
            ident_t = load_const(ident_d, [128, 128], f16)
            # additive mask (0 / -30000, f16-safe) computed host-side
            nmask = load_const(mask_d, [L, BC * L], f16)

            enc = cpool.tile([D, TREES], f16, tag="enc")  # enc^T, col = b*64+l
            pooled = cpool.tile([D, BC], f16, tag="pooled")

            idx_cols = NIDX_CHUNK // 16

            def tree_block(ebv, encb, nt):
                # Bottom-up subtree sums in the level-split layout: each
                # tree's 128 cols are [leaves(64) | lvl5(32) | ... | root | pad],
                # children of lvl-l node n sit at positions n and n+cnt of the
                # lvl-(l+1) region, so each level is two contiguous stride-1
                # half-adds (DVE 2x packed mode).
                for lvl in range(D_TREE - 2, -1, -1):
                    cnt = 2**lvl
                    sl = 128 - 2 ** (lvl + 1)
                    sc = 128 - 2 ** (lvl + 2)
                    nc.vector.tensor_add(
                        out=ebv[:, :, sl : sl + cnt],
                        in0=ebv[:, :, sl : sl + cnt],
                        in1=ebv[:, :, sc : sc + cnt],
                    )
                    nc.vector.tensor_add(
                        out=ebv[:, :, sl : sl + cnt],
                        in0=ebv[:, :, sl : sl + cnt],
                        in1=ebv[:, :, sc + cnt : sc + 2 * cnt],
                    )
                # node-max as pairwise halvings (2x mode; pad col duplicates
                # a leaf so the full 128-wide extent is safe to max over)
                mx = apool.tile([128, nt, 64], f16, tag=f"mx{nt}")
                nc.vector.tensor_max(mx[:], ebv[:, :, 0:64], ebv[:, :, 64:128])
                w = 32
                while w >= 2:
                    nc.vector.tensor_max(
                        mx[:, :, 0:w], mx[:, :, 0:w], mx[:, :, w : 2 * w]
                    )
                    w //= 2
                nc.vector.tensor_max(encb, mx[:, :, 0], mx[:, :, 1])
                # ReLU (max with the zero background) on the scalar engine
                nc.scalar.activation(encb, encb, mybir.ActivationFunctionType.Relu)

            for b in range(BC):
                eb = eblkpool.tile([128, L * 128], f16, tag="eb")
                ebv = eb.rearrange("p (t n) -> p t n", n=128)
                # last batch runs tree+max per 16-tree chunk so less of the
                # chain trails the final gathers
                fine = b >= BC - 1
                for k in range(CHUNKS_PER_BATCH):
                    c = b * CHUNKS_PER_BATCH + k
                    et = epool.tile([128, 1, NIDX_CHUNK], f16, tag="et")
                    last = c >= NCHUNKS - 4
                    nsub = 2 if last else 1
                    si = NIDX_CHUNK // nsub
                    sc_ = idx_cols // nsub
                    for sj in range(nsub):
                        nc.gpsimd.dma_gather(
                            et[:, :, sj * si : (sj + 1) * si],
                            emb_d[:],
                            idx_t[
                                :,
                                c * idx_cols + sj * sc_ : c * idx_cols + (sj + 1) * sc_,
                            ],
                            si,
                            si,
                            D,
                            transpose=True,
                            single_packet=False,
                            queue_num=(2 * (c % 2) + sj) if last else c % 4,
                        )
                    for j in range(NIDX_CHUNK // 512):
                        pp = tpsum.tile([128, 512], f32, tag="pp")
                        nc.tensor.matmul(
                            pp[:],
                            lhsT=wc_t[:],
                            rhs=et[:, 0, j * 512 : (j + 1) * 512],
                            start=True,
                            stop=True,
                        )
                        off = k * NIDX_CHUNK + j * 512
                        # PSUM->SBUF copy with the +bc bias folded in
                        nc.scalar.activation(
                            eb[:, off : off + 512],
                            pp[:],
                            mybir.ActivationFunctionType.Identity,
                            bias=bcv_t[:],
                            scale=1.0,
                        )
                    if fine:
                        t0 = k * CHUNK_TREES
                        tree_block(
                            ebv[:, t0 : t0 + CHUNK_TREES, :],
                            enc[:, b * L + t0 : b * L + t0 + CHUNK_TREES],
                            CHUNK_TREES,
                        )
                if not fine:
                    tree_block(ebv, enc[:, b * L : (b + 1) * L], L)
                encb = enc[:, b * L : (b + 1) * L]

                # ---- attention for this batch (Wq pre-scaled by 1/sqrt(D)) ----
                ab = apsum.tile([D, 512], f32, tag="ab")
                nc.tensor.matmul(ab[:, 0:L], lhsT=wq_t[:], rhs=encb, start=True, stop=True)
                nc.tensor.matmul(
                    ab[:, L : 2 * L], lhsT=wk_t[:], rhs=encb, start=True, stop=True
                )
                nc.tensor.matmul(
                    ab[:, 2 * L : 3 * L], lhsT=wv_t[:], rhs=encb, start=True, stop=True
                )
                qkvs = apool.tile([D, 3 * L], f16, tag="qkvs")
                nc.scalar.copy(out=qkvs[:], in_=ab[:, 0 : 3 * L])
                qs = qkvs[:, 0:L]
                ks = qkvs[:, L : 2 * L]
                vs = qkvs[:, 2 * L : 3 * L]

                sp = spsum.tile([L, 512], f32, tag="sp")
                nc.tensor.matmul(sp[:, 0:L], lhsT=qs, rhs=ks, start=True, stop=True)
                sm = apool.tile([L, L], f16, tag="sm")
                nc.vector.tensor_add(
                    out=sm[:], in0=sp[:, 0:L], in1=nmask[:, b * L : (b + 1) * L]
                )
                smv = sm.rearrange("q (o k) -> q o k", k=L)
                nrmax = apool.tile([L, 1], f32, tag="nrmax")
                nc.vector.reduce_max(
                    out=nrmax[:], in_=smv, axis=mybir.AxisListType.X, negate=True
                )
                # ex = exp(sm - rowmax) with the row sum accumulated in-pass
                ex = apool.tile([L, L], f32, tag="ex")
                rsum = apool.tile([L, 1], f32, tag="rsum")
                nc.scalar.activation(
                    ex[:],
                    sm[:],
                    mybir.ActivationFunctionType.Exp,
                    bias=nrmax[:],
                    scale=1.0,
                    accum_out=rsum[:],
                )
                rinv = apool.tile([L, 1], f32, tag="rinv")
                nc.vector.reciprocal(rinv[:], rsum[:])
                # attn = ex * rinv as a per-partition-scaled scalar-engine copy
                attn = apool.tile([L, L], f16, tag="attn")
                nc.scalar.activation(
                    attn[:],
                    ex[:],
                    mybir.ActivationFunctionType.Identity,
                    scale=rinv[:],
                )

                # transposes and attn@v (one fused PSUM->SBUF copy for both)
                spt = stsum.tile([L, 256], f16, tag="spt")
                nc.tensor.transpose(spt[:, 0:L], attn[:], ident_t[:L, :L])
                nc.tensor.transpose(spt[:, L : L + D], vs, ident_t[:])
                avs = apool.tile([L, 3 * L], f16, tag="avs")
                nc.scalar.copy(out=avs[:], in_=spt[:, 0 : 3 * L])
                ats = avs[:, 0:L]
                vts = avs[:, L : 3 * L]
                nc.tensor.matmul(
                    ab[:, 3 * L : 4 * L], lhsT=vts, rhs=ats, start=True, stop=True
                )
                os_t = apool.tile([D, L], f16, tag="os")
                nc.scalar.copy(out=os_t[:], in_=ab[:, 3 * L : 4 * L])
                nc.tensor.matmul(
                    ab[:, 4 * L : 5 * L], lhsT=wo_t[:], rhs=os_t[:], start=True, stop=True
                )
                nc.vector.reduce_max(
                    out=pooled[:, b : b + 1],
                    in_=ab[:, 4 * L : 5 * L].rearrange("d (o l) -> d o l", l=L),
                    axis=mybir.AxisListType.X,
                )

            # ---- logits ----
            lgp = spsum.tile([L, 512], f32, tag="sp")
            nc.tensor.matmul(
                lgp[0:BC, 0:LABELS], lhsT=pooled[:], rhs=wl_t[:], start=True, stop=True
            )
            outs = apool.tile([BC, LABELS], f32, tag="outs")
            nc.vector.tensor_add(out=outs[:], in0=lgp[0:BC, 0:LABELS], in1=blb_t[:])
            nc.sync.dma_start(out=out_d[:], in_=outs[:])

    nc.compile()
    return nc


def _get_nc():
    if "nc" not in _CACHE:
        _CACHE["nc"] = _build_nc()
    return _CACHE["nc"]


def kernel(tokens, mask, emb, Wc, bc, Wq, Wk, Wv, Wo, Wl, bl, _trace=False):
    from concourse.bass_utils import run_bass_kernel_spmd

    tokens = np.asarray(tokens)
    mask = np.asarray(mask)
    emb16 = np.asarray(emb, dtype=np.float32).astype(np.float16)
    emb16 = np.concatenate(
        [emb16, np.zeros((VPAD - VOCAB, D), np.float16)], axis=0
    )

    blb = np.tile(np.asarray(bl, np.float32)[None, :], (BC, 1))

    common = {
        "emb": emb16,
        "wc": np.asarray(Wc, np.float32),
        "bcv": np.asarray(bc, np.float32).reshape(D, 1),
        "wq": np.asarray(Wq, np.float32) / math.sqrt(float(D)),
        "wk": np.asarray(Wk, np.float32),
        "wv": np.asarray(Wv, np.float32),
        "wo": np.asarray(Wo, np.float32),
        "wl": np.asarray(Wl, np.float32),
        "blb": blb,
        "ident": np.eye(128, dtype=np.float32),
    }

    # level-split column order: [leaves(64) | lvl5(32) | ... | root], with
    # children of position n in level l at positions n and n+2**l of level l+1
    lvl_nodes = [[0]]
    for _ in range(D_TREE - 1):
        prev = lvl_nodes[-1]
        lvl_nodes.append([2 * k + 1 for k in prev] + [2 * k + 2 for k in prev])
    order = np.array([k for lvl in reversed(lvl_nodes) for k in lvl], np.int64)

    in_maps = []
    for c in range(NCORES):
        tok_c = np.asarray(tokens[c * BC : (c + 1) * BC]).reshape(TREES, NPT)
        tok_c = tok_c[:, order]
        # pad node duplicates a leaf: the max over the padded 128-wide
        # extent equals the max over the 127 real subtree-root snapshots
        idx_lin = np.concatenate([tok_c, tok_c[:, 0:1]], axis=1).reshape(-1)
        idx_arr = np.tile(
            idx_lin.astype(np.int16).reshape(-1, 16).T, (8, 1)
        )  # [128, NIDX_TOTAL/16]
        mask_c = (
            np.asarray(mask[c * BC : (c + 1) * BC], np.int32)
            .transpose(1, 0, 2)
            .reshape(L, BC * L)
        )
        mask_c = np.where(mask_c > 0, np.float16(0), np.float16(-30000))
        in_maps.append({**common, "idxs": idx_arr, "mask": mask_c})

    nc = _get_nc()
    res = run_bass_kernel_spmd(
        nc, in_maps, core_ids=list(range(NCORES)), trace=_trace
    )
    out = np.concatenate([r["out"] for r in res.results], axis=0)  # [B, LABELS]
    if _trace:
        return out, res
    return out
